# revision 6
# baseline (speedup 1.0000x reference)
"""Trainium2 Bass kernel for nn_AttentionAggregator.

Computation (per side, users/items symmetric):
    cu  = concat(gather(review_vecs, adj_r), gather(sec_vecs, adj_s))   # [6000, 1024]
    att = softmax(keys @ keys.T / 8) @ cu                               # [6000, 1024]
    out = relu(att @ W)                                                 # [6000, 1024]

Sharding: 8 cores run the same program (SPMD). Cores 0-3 take the user side
(1500 query rows each), cores 4-7 the item side. Keys, gather sources,
adjacency and weights are replicated; only the query slice differs.

On-device per core:
  - gather cu tile-by-tile from DRAM via indirect DMA (adjacency = row indices)
  - scoresT[k,q] = keys @ q.T via PE (contraction over D=64, zero-padded to 128)
  - E = exp(scoresT/8) on ScalarE directly PSUM->SBUF (no max-subtraction
    needed: |scores| <= ~14 in fp32)
  - O = E.T-weighted sum of cu, accumulated on PE in PSUM over chunks of
    6 k-tiles, then folded into an SBUF fp32 accumulator by DVE
  - rowsums r = E.T @ ones accumulated in a persistent PSUM bank
  - out = relu(O @ W) * (1/r), with the 1/r per-partition scale fused into
    the final ReLU PSUM->SBUF copy (valid since r > 0)

Column layout of the gathered cu is [review slots 0-7 | sec slots 0-7]
(instead of the reference's interleaved layout); the host permutes W's rows
to match, so results are identical.
"""

import os
import sys

import numpy as np

for _p in ("/opt/trn_rl_repo", "/root/.axon_site/_ro/trn_rl_repo"):
    if os.path.isdir(_p) and _p not in sys.path:
        sys.path.append(_p)

import concourse.bass as bass  # noqa: E402
import concourse.mybir as mybir  # noqa: E402
import concourse.tile as tile  # noqa: E402
from concourse import bacc  # noqa: E402
from concourse.bass_utils import run_bass_kernel_spmd  # noqa: E402
from concourse.masks import make_identity  # noqa: E402

P = 128
D = 64
NK = 6000          # keys per side
NKP = 6016         # padded to 47 full k-tiles
KT = NKP // P      # 47
QOUT = 1500        # query rows per core (6000 / 4 cores per side)
QP = 1536          # padded to 12 full q-subtiles
NQS = QP // P      # 12
HID = 1024
NR = 30000         # review_vecs rows
NS = 6000          # secondary source rows
KC = 6             # k-tiles per accumulation chunk
F32 = mybir.dt.float32
I32 = mybir.dt.int32

AF = mybir.ActivationFunctionType


def _emit_body(nc, tc, ctx_pools, tensors):
    """Emit one full pass of the kernel body inside an open TileContext."""
    from contextlib import ExitStack

    keysT, qvT, adj_r, adj_s, src_r, src_s, w, ebias, out = tensors
    const, psum, psum_r = ctx_pools

    # ---- persistent tiles -------------------------------------------------
    identity = const.tile([P, P], F32, tag="identity")
    make_identity(nc, identity[:])
    ones = const.tile([P, 1], F32, tag="ones")
    nc.gpsimd.memset(ones[:], 1.0)

    vecsT = const.tile([P, NKP], F32, tag="vecsT")
    nc.any.memzero(vecsT[D:, :])
    nc.sync.dma_start(vecsT[:D, :], keysT[:, :])

    qvT_sb = const.tile([P, QP], F32, tag="qvT")
    nc.any.memzero(qvT_sb[D:, :])
    nc.sync.dma_start(qvT_sb[:D, :], qvT[:, :])

    adj_r_sb = const.tile([P, KT, 8], I32, tag="adjr")
    nc.sync.dma_start(adj_r_sb[:], adj_r[:, :, :])
    adj_s_sb = const.tile([P, KT, 8], I32, tag="adjs")
    nc.sync.dma_start(adj_s_sb[:], adj_s[:, :, :])

    ebias_sb = const.tile([P, 1], F32, tag="ebias")
    nc.sync.dma_start(ebias_sb[:], ebias[:, :])

    o_acc = const.tile([P, NQS, HID], F32, tag="oacc")
    r_acc = const.tile([P, NQS], F32, tag="racc")
    rinv = const.tile([P, NQS], F32, tag="rinv")

    chunks = [list(range(c, min(c + KC, KT))) for c in range(0, KT, KC)]

    # ---- phase A: attention numerator + rowsums ---------------------------
    with ExitStack() as ctx:
        e_pool = ctx.enter_context(tc.tile_pool(name="e_pool", bufs=KC + 2))
        g_pool = ctx.enter_context(tc.tile_pool(name="g_pool", bufs=KC + 2))

        for ci, chunk in enumerate(chunks):
            first_chunk = ci == 0
            last_chunk = ci == len(chunks) - 1
            e_tiles = {}
            g_tiles = {}
            for kt in chunk:
                # HW indirect DMA consumes one index per partition row, so
                # gather one 64-wide neighbor slot per call (16 per k-tile).
                g = g_pool.tile([P, HID], F32, tag="g")
                for c in range(8):
                    nc.gpsimd.indirect_dma_start(
                        out=g[:, c * D:(c + 1) * D],
                        out_offset=None,
                        in_=src_r[:],
                        in_offset=bass.IndirectOffsetOnAxis(
                            ap=adj_r_sb[:, kt, c:c + 1], axis=0),
                    )
                    nc.gpsimd.indirect_dma_start(
                        out=g[:, 512 + c * D:512 + (c + 1) * D],
                        out_offset=None,
                        in_=src_s[:],
                        in_offset=bass.IndirectOffsetOnAxis(
                            ap=adj_s_sb[:, kt, c:c + 1], axis=0),
                    )
                g_tiles[kt] = g

                e = e_pool.tile([P, QP], F32, tag="e")
                lhsT = vecsT[:, kt * P:(kt + 1) * P]
                for i in range(QP // 512):
                    s_ps = psum.tile([P, 512], F32, tag="ps")
                    nc.tensor.matmul(
                        s_ps[:], lhsT, qvT_sb[:, i * 512:(i + 1) * 512],
                        start=True, stop=True,
                    )
                    # padded key rows (6000..6015) get bias -1e30 so
                    # exp() forces their attention weight to exactly zero
                    bias = ebias_sb[:, 0:1] if kt == KT - 1 else 0.0
                    nc.scalar.activation(
                        e[:, i * 512:(i + 1) * 512], s_ps[:], AF.Exp,
                        bias=bias, scale=0.125,
                    )
                e_tiles[kt] = e

            r_ps = psum_r.tile([P, NQS], F32, tag="rps")
            for j in range(NQS):
                p0 = psum.tile([P, 512], F32, tag="ps")
                p1 = psum.tile([P, 512], F32, tag="ps")
                for i, kt in enumerate(chunk):
                    lhsT = e_tiles[kt][:, j * P:(j + 1) * P]
                    first = i == 0
                    last = i == len(chunk) - 1
                    nc.tensor.matmul(p0[:], lhsT, g_tiles[kt][:, 0:512],
                                     start=first, stop=last)
                    nc.tensor.matmul(p1[:], lhsT, g_tiles[kt][:, 512:1024],
                                     start=first, stop=last)
                    nc.tensor.matmul(r_ps[:, j:j + 1], lhsT, ones[:],
                                     start=first, stop=last)
                if first_chunk:
                    nc.vector.tensor_copy(o_acc[:, j, 0:512], p0[:])
                    nc.vector.tensor_copy(o_acc[:, j, 512:1024], p1[:])
                else:
                    nc.vector.tensor_add(o_acc[:, j, 0:512], o_acc[:, j, 0:512], p0[:])
                    nc.vector.tensor_add(o_acc[:, j, 512:1024], o_acc[:, j, 512:1024], p1[:])
            if first_chunk:
                nc.vector.tensor_copy(r_acc[:], r_ps[:])
            else:
                nc.vector.tensor_add(r_acc[:], r_acc[:], r_ps[:])

    # ---- phase B: normalize (folded), project through W, relu, store ------
    nc.vector.reciprocal(rinv[:], r_acc[:])

    with ExitStack() as ctx:
        w_pool = ctx.enter_context(tc.tile_pool(name="w_pool", bufs=1))
        ot_pool = ctx.enter_context(tc.tile_pool(name="ot_pool", bufs=10))
        ob_pool = ctx.enter_context(tc.tile_pool(name="ob_pool", bufs=4))

        w_sb = w_pool.tile([P, HID // P, HID], F32, tag="w")
        nc.sync.dma_start(w_sb[:], w[:, :, :])

        for j in range(NQS):
            ots = []
            for t in range(HID // P):
                tp = psum.tile([P, 512], F32, tag="ps")
                nc.tensor.transpose(
                    tp[:, 0:P], o_acc[:, j, t * P:(t + 1) * P], identity[:],
                )
                ot = ot_pool.tile([P, P], F32, tag="ot")
                nc.vector.tensor_copy(ot[:], tp[:, 0:P])
                ots.append(ot)
            for h in range(HID // 512):
                pf = psum.tile([P, 512], F32, tag="ps")
                for t in range(HID // P):
                    nc.tensor.matmul(
                        pf[:], ots[t][:], w_sb[:, t, h * 512:(h + 1) * 512],
                        start=(t == 0), stop=(t == HID // P - 1),
                    )
                ob = ob_pool.tile([P, 512], F32, tag="ob")
                nc.scalar.activation(ob[:], pf[:], AF.Relu, scale=rinv[:, j:j + 1])
                rows = min(P, QOUT - j * P)
                if rows > 0:
                    nc.sync.dma_start(
                        out[j * P:j * P + rows, h * 512:(h + 1) * 512], ob[:rows, :],
                    )


def build_program(repeat: int = 0):
    """Build + compile the SPMD program. repeat>0 wraps the body in a
    device-side For loop (for timing) and is not used for grading."""
    from contextlib import ExitStack

    nc = bacc.Bacc("TRN2", target_bir_lowering=False, debug=False, num_devices=8)

    keysT = nc.dram_tensor("keysT", [D, NKP], F32, kind="ExternalInput")
    qvT = nc.dram_tensor("qvT", [D, QP], F32, kind="ExternalInput")
    adj_r = nc.dram_tensor("adj_r", [P, KT, 8], I32, kind="ExternalInput")
    adj_s = nc.dram_tensor("adj_s", [P, KT, 8], I32, kind="ExternalInput")
    src_r = nc.dram_tensor("src_r", [NR, D], F32, kind="ExternalInput")
    src_s = nc.dram_tensor("src_s", [NS, D], F32, kind="ExternalInput")
    w = nc.dram_tensor("w", [P, HID // P, HID], F32, kind="ExternalInput")
    ebias = nc.dram_tensor("ebias", [P, 1], F32, kind="ExternalInput")
    out = nc.dram_tensor("out", [QOUT, HID], F32, kind="ExternalOutput")

    tensors = (keysT, qvT, adj_r, adj_s, src_r, src_s, w, ebias, out)

    with tile.TileContext(nc) as tc, ExitStack() as ctx:
        const = ctx.enter_context(tc.tile_pool(name="const", bufs=1))
        psum = ctx.enter_context(tc.tile_pool(name="psum", bufs=6, space="PSUM"))
        psum_r = ctx.enter_context(tc.tile_pool(name="psum_r", bufs=2, space="PSUM"))
        pools = (const, psum, psum_r)
        if repeat > 0:
            with tc.For_i(0, repeat, 1):
                _emit_body(nc, tc, pools, tensors)
        else:
            _emit_body(nc, tc, pools, tensors)

    nc.compile()
    return nc


def _permute_w(w_full: np.ndarray) -> np.ndarray:
    """Reference cu columns are slot-interleaved [r0 i0 r1 i1 ...]; the kernel
    gathers [r0..r7 | i0..i7]. Permute W rows to match, then pre-tile to
    [128, 8, 1024] for the on-device layout."""
    wr = w_full.reshape(8, 2, D, HID)
    w_perm = np.concatenate(
        [wr[:, 0].reshape(8 * D, HID), wr[:, 1].reshape(8 * D, HID)], axis=0,
    )
    return np.ascontiguousarray(
        w_perm.reshape(HID // P, P, HID).transpose(1, 0, 2),
    )


def _pad_adj(adj: np.ndarray) -> np.ndarray:
    """[6000, 8] -> [128, 47, 8] int32 with padded rows indexing row 0
    (harmless: their attention weight is forced to zero on device)."""
    a = np.zeros((NKP, 8), dtype=np.int32)
    a[:NK] = adj
    return np.ascontiguousarray(a.reshape(KT, P, 8).transpose(1, 0, 2))


def _host_inputs(review_vecs, user_vecs, item_vecs, user_weights, item_weights,
                 user_review_adj, user_item_adj, item_review_adj, item_user_adj):
    review_vecs = np.asarray(review_vecs, dtype=np.float32)
    user_vecs = np.asarray(user_vecs, dtype=np.float32)
    item_vecs = np.asarray(item_vecs, dtype=np.float32)

    sides = {}
    for side, keys, adj_r, adj_s, src_s, w_full in (
        ("user", user_vecs, user_review_adj, user_item_adj, item_vecs, user_weights),
        ("item", item_vecs, item_review_adj, item_user_adj, user_vecs, item_weights),
    ):
        keysT = np.zeros((D, NKP), dtype=np.float32)
        keysT[:, :NK] = keys.T
        sides[side] = dict(
            keysT=keysT,
            adj_r=_pad_adj(np.asarray(adj_r, dtype=np.int32)),
            adj_s=_pad_adj(np.asarray(adj_s, dtype=np.int32)),
            src_s=np.ascontiguousarray(src_s),
            w=_permute_w(np.asarray(w_full, dtype=np.float32)),
            keys=keys,
        )

    ebias = np.zeros((P, 1), dtype=np.float32)
    ebias[NK - (KT - 1) * P:] = -1e30

    in_maps = []
    for c in range(8):
        s = sides["user" if c < 4 else "item"]
        b = c % 4
        qv = s["keys"][b * QOUT:(b + 1) * QOUT]  # [1500, 64]
        qvT = np.empty((D, QP), dtype=np.float32)
        qvT[:, :QOUT] = qv.T
        qvT[:, QOUT:] = qv.T[:, :QP - QOUT]  # pad with real vectors (finite rowsums)
        in_maps.append(dict(
            keysT=s["keysT"], qvT=np.ascontiguousarray(qvT),
            adj_r=s["adj_r"], adj_s=s["adj_s"],
            src_r=review_vecs, src_s=s["src_s"], w=s["w"], ebias=ebias,
        ))
    return in_maps


_NC_CACHE = None


def kernel(**inputs):
    global _NC_CACHE
    if _NC_CACHE is None:
        _NC_CACHE = build_program()
    nc = _NC_CACHE
    in_maps = _host_inputs(**inputs)
    res = run_bass_kernel_spmd(nc, in_maps, core_ids=list(range(8)))
    outs = [res.results[c]["out"] for c in range(8)]
    user_output = np.concatenate(outs[0:4], axis=0)
    item_output = np.concatenate(outs[4:8], axis=0)
    return user_output, item_output


# revision 9
# speedup vs baseline: 1.1840x; 1.1840x over previous
"""Trainium2 Bass kernel for nn_AttentionAggregator.

Computation (per side, users/items symmetric):
    cu  = concat(gather(review_vecs, adj_r), gather(sec_vecs, adj_s))   # [6000, 1024]
    att = softmax(keys @ keys.T / 8) @ cu                               # [6000, 1024]
    out = relu(att @ W)                                                 # [6000, 1024]

Sharding: 8 cores run the same program (SPMD). Cores 0-3 take the user side
(1500 query rows each), cores 4-7 the item side. Keys, gather sources,
adjacency and weights are replicated; only the query slice differs.

On-device per core:
  - gather cu tile-by-tile from DRAM via indirect DMA (adjacency = row indices)
  - scoresT[k,q] = keys @ q.T via PE (contraction over D=64, zero-padded to 128)
  - E = exp(scoresT/8) on ScalarE directly PSUM->SBUF (no max-subtraction
    needed: |scores| <= ~14 in fp32)
  - O = E.T-weighted sum of cu, accumulated on PE in PSUM over chunks of
    6 k-tiles, then folded into an SBUF fp32 accumulator by DVE
  - rowsums r = E.T @ ones accumulated in a persistent PSUM bank
  - out = relu(O @ W) * (1/r), with the 1/r per-partition scale fused into
    the final ReLU PSUM->SBUF copy (valid since r > 0)

Column layout of the gathered cu is [review slots 0-7 | sec slots 0-7]
(instead of the reference's interleaved layout); the host permutes W's rows
to match, so results are identical.
"""

import os
import sys

import numpy as np

for _p in ("/opt/trn_rl_repo", "/root/.axon_site/_ro/trn_rl_repo"):
    if os.path.isdir(_p) and _p not in sys.path:
        sys.path.append(_p)

import concourse.bass as bass  # noqa: E402
import concourse.mybir as mybir  # noqa: E402
import concourse.tile as tile  # noqa: E402
from concourse import bacc  # noqa: E402
from concourse.bass_utils import run_bass_kernel_spmd  # noqa: E402
from concourse.masks import make_identity  # noqa: E402

P = 128
D = 64
NK = 6000          # keys per side
NKP = 6016         # padded to 47 full k-tiles
KT = NKP // P      # 47
QOUT = 1500        # query rows per core (6000 / 4 cores per side)
QP = 1536          # padded to 12 full q-subtiles
NQS = QP // P      # 12
HID = 1024
NR = 30000         # review_vecs rows
NS = 6000          # secondary source rows
KC = 6             # k-tiles per accumulation chunk
F32 = mybir.dt.float32
I32 = mybir.dt.int32

AF = mybir.ActivationFunctionType


GATHER_MODE = "indirect"  # "indirect" | "fake" (timing experiments only)


def _emit_body(nc, tc, ctx_pools, tensors):
    """Emit one full pass of the kernel body inside an open TileContext."""
    from contextlib import ExitStack

    keysT, qvT, adj_r, adj_s, src_r, src_s, w, ebias, out = tensors
    const, psum, psum_r = ctx_pools

    # ---- persistent tiles -------------------------------------------------
    identity = const.tile([P, P], F32, tag="identity")
    make_identity(nc, identity[:])
    ones = const.tile([P, 1], F32, tag="ones")
    nc.gpsimd.memset(ones[:], 1.0)

    vecsT = const.tile([P, NKP], F32, tag="vecsT")
    nc.any.memzero(vecsT[D:, :])
    nc.sync.dma_start(vecsT[:D, :], keysT[:, :])

    qvT_sb = const.tile([P, QP], F32, tag="qvT")
    nc.any.memzero(qvT_sb[D:, :])
    nc.sync.dma_start(qvT_sb[:D, :], qvT[:, :])

    adj_r_sb = const.tile([P, KT, 8], I32, tag="adjr")
    nc.sync.dma_start(adj_r_sb[:], adj_r[:, :, :])
    adj_s_sb = const.tile([P, KT, 8], I32, tag="adjs")
    nc.sync.dma_start(adj_s_sb[:], adj_s[:, :, :])

    ebias_sb = const.tile([P, 1], F32, tag="ebias")
    nc.sync.dma_start(ebias_sb[:], ebias[:, :])

    o_acc = const.tile([P, NQS, HID], F32, tag="oacc")
    r_acc = const.tile([P, NQS], F32, tag="racc")
    rinv = const.tile([P, NQS], F32, tag="rinv")

    chunks = [list(range(c, min(c + KC, KT))) for c in range(0, KT, KC)]

    # ---- phase A: attention numerator + rowsums ---------------------------
    with ExitStack() as ctx:
        e_pool = ctx.enter_context(tc.tile_pool(name="e_pool", bufs=KC + 2))
        g_pool = ctx.enter_context(tc.tile_pool(name="g_pool", bufs=KC + 2))

        for ci, chunk in enumerate(chunks):
            first_chunk = ci == 0
            last_chunk = ci == len(chunks) - 1
            e_tiles = {}
            g_tiles = {}
            for kt in chunk:
                # HW indirect DMA consumes one index per partition row, so
                # gather one 64-wide neighbor slot per call (16 per k-tile).
                g = g_pool.tile([P, HID], F32, tag="g")
                if GATHER_MODE == "fake":
                    # contiguous stand-in load of the same volume (timing only)
                    nc.sync.dma_start(
                        g[:, 0:512],
                        src_r[(kt % 29) * 8 * P:((kt % 29) * 8 + 8) * P, :]
                        .rearrange("(a b) e -> a (b e)", b=8))
                    nc.sync.dma_start(
                        g[:, 512:1024],
                        src_s[(kt % 5) * 8 * P:((kt % 5) * 8 + 8) * P, :]
                        .rearrange("(a b) e -> a (b e)", b=8))
                else:
                    for c in range(8):
                        nc.gpsimd.indirect_dma_start(
                            out=g[:, c * D:(c + 1) * D],
                            out_offset=None,
                            in_=src_r[:],
                            in_offset=bass.IndirectOffsetOnAxis(
                                ap=adj_r_sb[:, kt, c:c + 1], axis=0),
                        )
                        nc.gpsimd.indirect_dma_start(
                            out=g[:, 512 + c * D:512 + (c + 1) * D],
                            out_offset=None,
                            in_=src_s[:],
                            in_offset=bass.IndirectOffsetOnAxis(
                                ap=adj_s_sb[:, kt, c:c + 1], axis=0),
                        )
                g_tiles[kt] = g

                e = e_pool.tile([P, QP], F32, tag="e")
                lhsT = vecsT[:, kt * P:(kt + 1) * P]
                for i in range(QP // 512):
                    s_ps = psum.tile([P, 512], F32, tag="ps")
                    nc.tensor.matmul(
                        s_ps[:], lhsT, qvT_sb[:, i * 512:(i + 1) * 512],
                        start=True, stop=True,
                    )
                    # padded key rows (6000..6015) get bias -1e30 so
                    # exp() forces their attention weight to exactly zero
                    bias = ebias_sb[:, 0:1] if kt == KT - 1 else 0.0
                    nc.scalar.activation(
                        e[:, i * 512:(i + 1) * 512], s_ps[:], AF.Exp,
                        bias=bias, scale=0.125,
                    )
                e_tiles[kt] = e

            r_ps = psum_r.tile([P, NQS], F32, tag="rps")
            for j in range(NQS):
                p0 = psum.tile([P, 512], F32, tag="ps")
                p1 = psum.tile([P, 512], F32, tag="ps")
                for i, kt in enumerate(chunk):
                    lhsT = e_tiles[kt][:, j * P:(j + 1) * P]
                    first = i == 0
                    last = i == len(chunk) - 1
                    nc.tensor.matmul(p0[:], lhsT, g_tiles[kt][:, 0:512],
                                     start=first, stop=last)
                    nc.tensor.matmul(p1[:], lhsT, g_tiles[kt][:, 512:1024],
                                     start=first, stop=last)
                    nc.tensor.matmul(r_ps[:, j:j + 1], lhsT, ones[:],
                                     start=first, stop=last)
                if first_chunk:
                    nc.vector.tensor_copy(o_acc[:, j, 0:512], p0[:])
                    nc.vector.tensor_copy(o_acc[:, j, 512:1024], p1[:])
                else:
                    nc.vector.tensor_add(o_acc[:, j, 0:512], o_acc[:, j, 0:512], p0[:])
                    nc.vector.tensor_add(o_acc[:, j, 512:1024], o_acc[:, j, 512:1024], p1[:])
            if first_chunk:
                nc.vector.tensor_copy(r_acc[:], r_ps[:])
            else:
                nc.vector.tensor_add(r_acc[:], r_acc[:], r_ps[:])

    # ---- phase B: normalize (folded), project through W, relu, store ------
    nc.vector.reciprocal(rinv[:], r_acc[:])

    with ExitStack() as ctx:
        w_pool = ctx.enter_context(tc.tile_pool(name="w_pool", bufs=1))
        ot_pool = ctx.enter_context(tc.tile_pool(name="ot_pool", bufs=10))
        ob_pool = ctx.enter_context(tc.tile_pool(name="ob_pool", bufs=4))

        w_sb = w_pool.tile([P, HID // P, HID], F32, tag="w")
        nc.sync.dma_start(w_sb[:], w[:, :, :])

        for j in range(NQS):
            ots = []
            for t in range(HID // P):
                tp = psum.tile([P, 512], F32, tag="ps")
                nc.tensor.transpose(
                    tp[:, 0:P], o_acc[:, j, t * P:(t + 1) * P], identity[:],
                )
                ot = ot_pool.tile([P, P], F32, tag="ot")
                nc.vector.tensor_copy(ot[:], tp[:, 0:P])
                ots.append(ot)
            for h in range(HID // 512):
                pf = psum.tile([P, 512], F32, tag="ps")
                for t in range(HID // P):
                    nc.tensor.matmul(
                        pf[:], ots[t][:], w_sb[:, t, h * 512:(h + 1) * 512],
                        start=(t == 0), stop=(t == HID // P - 1),
                    )
                ob = ob_pool.tile([P, 512], F32, tag="ob")
                nc.scalar.activation(ob[:], pf[:], AF.Relu, scale=rinv[:, j:j + 1])
                rows = min(P, QOUT - j * P)
                if rows > 0:
                    nc.sync.dma_start(
                        out[j * P:j * P + rows, h * 512:(h + 1) * 512], ob[:rows, :],
                    )


def build_program(repeat: int = 0, scratch: int | None = None):
    """Build + compile the SPMD program. repeat>0 wraps the body in a
    device-side For loop (for timing) and is not used for grading."""
    from contextlib import ExitStack

    kw = {} if scratch is None else dict(dynamic_dma_scratch_size=scratch)
    nc = bacc.Bacc("TRN2", target_bir_lowering=False, debug=False, num_devices=8, **kw)

    keysT = nc.dram_tensor("keysT", [D, NKP], F32, kind="ExternalInput")
    qvT = nc.dram_tensor("qvT", [D, QP], F32, kind="ExternalInput")
    adj_r = nc.dram_tensor("adj_r", [P, KT, 8], I32, kind="ExternalInput")
    adj_s = nc.dram_tensor("adj_s", [P, KT, 8], I32, kind="ExternalInput")
    src_r = nc.dram_tensor("src_r", [NR, D], F32, kind="ExternalInput")
    src_s = nc.dram_tensor("src_s", [NS, D], F32, kind="ExternalInput")
    w = nc.dram_tensor("w", [P, HID // P, HID], F32, kind="ExternalInput")
    ebias = nc.dram_tensor("ebias", [P, 1], F32, kind="ExternalInput")
    out = nc.dram_tensor("out", [QOUT, HID], F32, kind="ExternalOutput")

    tensors = (keysT, qvT, adj_r, adj_s, src_r, src_s, w, ebias, out)

    with tile.TileContext(nc) as tc, ExitStack() as ctx:
        const = ctx.enter_context(tc.tile_pool(name="const", bufs=1))
        psum = ctx.enter_context(tc.tile_pool(name="psum", bufs=6, space="PSUM"))
        psum_r = ctx.enter_context(tc.tile_pool(name="psum_r", bufs=2, space="PSUM"))
        pools = (const, psum, psum_r)
        if repeat > 0:
            with tc.For_i(0, repeat, 1):
                _emit_body(nc, tc, pools, tensors)
        else:
            _emit_body(nc, tc, pools, tensors)

    nc.compile()
    return nc


def _permute_w(w_full: np.ndarray) -> np.ndarray:
    """Reference cu columns are slot-interleaved [r0 i0 r1 i1 ...]; the kernel
    gathers [r0..r7 | i0..i7]. Permute W rows to match, then pre-tile to
    [128, 8, 1024] for the on-device layout."""
    wr = w_full.reshape(8, 2, D, HID)
    w_perm = np.concatenate(
        [wr[:, 0].reshape(8 * D, HID), wr[:, 1].reshape(8 * D, HID)], axis=0,
    )
    return np.ascontiguousarray(
        w_perm.reshape(HID // P, P, HID).transpose(1, 0, 2),
    )


def _pad_adj(adj: np.ndarray) -> np.ndarray:
    """[6000, 8] -> [128, 47, 8] int32 with padded rows indexing row 0
    (harmless: their attention weight is forced to zero on device)."""
    a = np.zeros((NKP, 8), dtype=np.int32)
    a[:NK] = adj
    return np.ascontiguousarray(a.reshape(KT, P, 8).transpose(1, 0, 2))


def _host_inputs(review_vecs, user_vecs, item_vecs, user_weights, item_weights,
                 user_review_adj, user_item_adj, item_review_adj, item_user_adj):
    review_vecs = np.asarray(review_vecs, dtype=np.float32)
    user_vecs = np.asarray(user_vecs, dtype=np.float32)
    item_vecs = np.asarray(item_vecs, dtype=np.float32)

    sides = {}
    for side, keys, adj_r, adj_s, src_s, w_full in (
        ("user", user_vecs, user_review_adj, user_item_adj, item_vecs, user_weights),
        ("item", item_vecs, item_review_adj, item_user_adj, user_vecs, item_weights),
    ):
        keysT = np.zeros((D, NKP), dtype=np.float32)
        keysT[:, :NK] = keys.T
        sides[side] = dict(
            keysT=keysT,
            adj_r=_pad_adj(np.asarray(adj_r, dtype=np.int32)),
            adj_s=_pad_adj(np.asarray(adj_s, dtype=np.int32)),
            src_s=np.ascontiguousarray(src_s),
            w=_permute_w(np.asarray(w_full, dtype=np.float32)),
            keys=keys,
        )

    ebias = np.zeros((P, 1), dtype=np.float32)
    ebias[NK - (KT - 1) * P:] = -1e30

    in_maps = []
    for c in range(8):
        s = sides["user" if c < 4 else "item"]
        b = c % 4
        qv = s["keys"][b * QOUT:(b + 1) * QOUT]  # [1500, 64]
        qvT = np.empty((D, QP), dtype=np.float32)
        qvT[:, :QOUT] = qv.T
        qvT[:, QOUT:] = qv.T[:, :QP - QOUT]  # pad with real vectors (finite rowsums)
        in_maps.append(dict(
            keysT=s["keysT"], qvT=np.ascontiguousarray(qvT),
            adj_r=s["adj_r"], adj_s=s["adj_s"],
            src_r=review_vecs, src_s=s["src_s"], w=s["w"], ebias=ebias,
        ))
    return in_maps


_NC_CACHE = None


def kernel(**inputs):
    global _NC_CACHE
    if _NC_CACHE is None:
        _NC_CACHE = build_program()
    nc = _NC_CACHE
    in_maps = _host_inputs(**inputs)
    res = run_bass_kernel_spmd(nc, in_maps, core_ids=list(range(8)))
    outs = [res.results[c]["out"] for c in range(8)]
    user_output = np.concatenate(outs[0:4], axis=0)
    item_output = np.concatenate(outs[4:8], axis=0)
    return user_output, item_output


# revision 13
# speedup vs baseline: 1.1945x; 1.0089x over previous
"""Trainium2 Bass kernel for nn_AttentionAggregator.

Computation (per side, users/items symmetric):
    cu  = concat(gather(review_vecs, adj_r), gather(sec_vecs, adj_s))   # [6000, 1024]
    att = softmax(keys @ keys.T / 8) @ cu                               # [6000, 1024]
    out = relu(att @ W)                                                 # [6000, 1024]

Sharding: 8 cores run the same program (SPMD). Cores 0-3 take the user side
(1500 query rows each), cores 4-7 the item side. Keys, gather sources,
adjacency and weights are replicated; only the query slice differs.

On-device per core:
  - gather cu tile-by-tile from DRAM via indirect DMA (adjacency = row indices)
  - scoresT[k,q] = keys @ q.T via PE (contraction over D=64, zero-padded to 128)
  - E = exp(scoresT/8) on ScalarE directly PSUM->SBUF (no max-subtraction
    needed: |scores| <= ~14 in fp32)
  - O = E.T-weighted sum of cu, accumulated on PE in PSUM over chunks of
    6 k-tiles, then folded into an SBUF fp32 accumulator by DVE
  - rowsums r = E.T @ ones accumulated in a persistent PSUM bank
  - out = relu(O @ W) * (1/r), with the 1/r per-partition scale fused into
    the final ReLU PSUM->SBUF copy (valid since r > 0)

Column layout of the gathered cu is [review slots 0-7 | sec slots 0-7]
(instead of the reference's interleaved layout); the host permutes W's rows
to match, so results are identical.
"""

import os
import sys

import ml_dtypes
import numpy as np

for _p in ("/opt/trn_rl_repo", "/root/.axon_site/_ro/trn_rl_repo"):
    if os.path.isdir(_p) and _p not in sys.path:
        sys.path.append(_p)

import concourse.bass as bass  # noqa: E402
import concourse.mybir as mybir  # noqa: E402
import concourse.tile as tile  # noqa: E402
from concourse import bacc  # noqa: E402
from concourse.bass_utils import run_bass_kernel_spmd  # noqa: E402
from concourse.masks import make_identity  # noqa: E402

P = 128
D = 64
NK = 6000          # keys per side
NKP = 6144         # padded to 48 full k-tiles (6 allgather chunks of 8)
KT = NKP // P      # 48
KT_CALC = 47       # k-tiles that carry real keys (kt 47 is all padding)
QOUT = 1500        # query rows per core (6000 / 4 cores per side)
QP = 1536          # padded to 12 full q-subtiles
NQS = QP // P      # 12
HID = 1024
NR = 30000         # review_vecs rows
NS = 6000          # secondary source rows
KC = 8             # k-tiles per chunk (= one allgather round)
GRP = 4            # cores per attention side (allgather group size)
LOC = KC // GRP    # k-tiles gathered locally per core per chunk (2)
F32 = mybir.dt.float32
BF16 = mybir.dt.bfloat16
I32 = mybir.dt.int32

AF = mybir.ActivationFunctionType


GATHER_MODE = "indirect"  # "indirect" | "fake" (timing experiments only)


def _emit_body(nc, tc, ctx_pools, tensors):
    """Emit one full pass of the kernel body inside an open TileContext."""
    from contextlib import ExitStack

    keysT, qvT, adj_r, adj_s, src_r, src_s, w, ebias, out = tensors
    const, psum, psum_r = ctx_pools

    # ---- persistent tiles -------------------------------------------------
    identity = const.tile([P, P], F32, tag="identity")
    make_identity(nc, identity[:])
    ones = const.tile([P, 1], BF16, tag="ones")
    nc.gpsimd.memset(ones[:], 1.0)

    vecsT = const.tile([P, NKP], F32, tag="vecsT")
    nc.any.memzero(vecsT[D:, :])
    nc.sync.dma_start(vecsT[:D, :], keysT[:, :])

    qvT_sb = const.tile([P, QP], F32, tag="qvT")
    nc.any.memzero(qvT_sb[D:, :])
    nc.sync.dma_start(qvT_sb[:D, :], qvT[:, :])

    adj_r_sb = const.tile([P, KT // GRP, 8], I32, tag="adjr")
    nc.sync.dma_start(adj_r_sb[:], adj_r[:, :, :])
    adj_s_sb = const.tile([P, KT // GRP, 8], I32, tag="adjs")
    nc.sync.dma_start(adj_s_sb[:], adj_s[:, :, :])

    ebias_sb = const.tile([P, 1], F32, tag="ebias")
    nc.sync.dma_start(ebias_sb[:], ebias[:, :])

    o_acc = const.tile([P, NQS, HID], F32, tag="oacc")
    r_acc = const.tile([P, NQS], F32, tag="racc")
    rinv = const.tile([P, NQS], F32, tag="rinv")

    chunks = [list(range(c, min(c + KC, KT_CALC))) for c in range(0, KT, KC)]

    # ---- phase A: attention numerator + rowsums ---------------------------
    # Each core indirect-gathers only LOC k-tiles per chunk (its rank's
    # share, selected by the per-core adjacency data), then the 4 cores of
    # the side exchange their shares with one AllGather per chunk.
    with ExitStack() as ctx:
        e_pool = ctx.enter_context(tc.tile_pool(name="e_pool", bufs=KC + 2))
        g_pool = ctx.enter_context(tc.tile_pool(name="g_pool", bufs=KC + 2))
        gl_pool = ctx.enter_context(tc.tile_pool(name="gl_pool", bufs=2 * LOC))
        dram = ctx.enter_context(tc.tile_pool(name="dram", bufs=2, space="DRAM"))

        for ci, chunk in enumerate(chunks):
            first_chunk = ci == 0
            last_chunk = ci == len(chunks) - 1
            e_tiles = {}
            g_tiles = {}

            # local gathers (one index per partition row per call)
            bin_ = dram.tile([LOC * P, HID], BF16, tag="bin")
            for i in range(LOC):
                gl = gl_pool.tile([P, HID], BF16, tag="gl")
                for c in range(8):
                    nc.gpsimd.indirect_dma_start(
                        out=gl[:, c * D:(c + 1) * D],
                        out_offset=None,
                        in_=src_r[:],
                        in_offset=bass.IndirectOffsetOnAxis(
                            ap=adj_r_sb[:, ci * LOC + i, c:c + 1], axis=0),
                    )
                    nc.gpsimd.indirect_dma_start(
                        out=gl[:, 512 + c * D:512 + (c + 1) * D],
                        out_offset=None,
                        in_=src_s[:],
                        in_offset=bass.IndirectOffsetOnAxis(
                            ap=adj_s_sb[:, ci * LOC + i, c:c + 1], axis=0),
                    )
                nc.sync.dma_start(bin_[i * P:(i + 1) * P, :], gl[:])

            bout = dram.tile([KC * P, HID], BF16, tag="bout")
            nc.gpsimd.collective_compute(
                "AllGather",
                mybir.AluOpType.bypass,
                replica_groups=[[0, 1, 2, 3], [4, 5, 6, 7]],
                ins=[bin_.opt()],
                outs=[bout.opt()],
            )

            for t, kt in enumerate(chunk):
                g = g_pool.tile([P, HID], BF16, tag="g")
                nc.sync.dma_start(g[:], bout[t * P:(t + 1) * P, :])
                g_tiles[kt] = g

                e = e_pool.tile([P, QP], BF16, tag="e")
                lhsT = vecsT[:, kt * P:(kt + 1) * P]
                for i in range(QP // 512):
                    s_ps = psum.tile([P, 512], F32, tag="ps")
                    nc.tensor.matmul(
                        s_ps[:], lhsT, qvT_sb[:, i * 512:(i + 1) * 512],
                        start=True, stop=True,
                    )
                    # padded key rows (6000..6015) get bias -1e30 so
                    # exp() forces their attention weight to exactly zero
                    bias = ebias_sb[:, 0:1] if kt == KT_CALC - 1 else 0.0
                    nc.scalar.activation(
                        e[:, i * 512:(i + 1) * 512], s_ps[:], AF.Exp,
                        bias=bias, scale=0.125,
                    )
                e_tiles[kt] = e

            r_ps = psum_r.tile([P, NQS], F32, tag="rps")
            for j in range(NQS):
                p0 = psum.tile([P, 512], F32, tag="ps")
                p1 = psum.tile([P, 512], F32, tag="ps")
                for i, kt in enumerate(chunk):
                    lhsT = e_tiles[kt][:, j * P:(j + 1) * P]
                    first = i == 0
                    last = i == len(chunk) - 1
                    nc.tensor.matmul(p0[:], lhsT, g_tiles[kt][:, 0:512],
                                     start=first, stop=last)
                    nc.tensor.matmul(p1[:], lhsT, g_tiles[kt][:, 512:1024],
                                     start=first, stop=last)
                    nc.tensor.matmul(r_ps[:, j:j + 1], lhsT, ones[:],
                                     start=first, stop=last)
                if first_chunk:
                    nc.vector.tensor_copy(o_acc[:, j, 0:512], p0[:])
                    nc.vector.tensor_copy(o_acc[:, j, 512:1024], p1[:])
                else:
                    nc.vector.tensor_add(o_acc[:, j, 0:512], o_acc[:, j, 0:512], p0[:])
                    nc.vector.tensor_add(o_acc[:, j, 512:1024], o_acc[:, j, 512:1024], p1[:])
            if first_chunk:
                nc.vector.tensor_copy(r_acc[:], r_ps[:])
            else:
                nc.vector.tensor_add(r_acc[:], r_acc[:], r_ps[:])

    # ---- phase B: normalize (folded), project through W, relu, store ------
    nc.vector.reciprocal(rinv[:], r_acc[:])

    with ExitStack() as ctx:
        w_pool = ctx.enter_context(tc.tile_pool(name="w_pool", bufs=1))
        ot_pool = ctx.enter_context(tc.tile_pool(name="ot_pool", bufs=10))
        ob_pool = ctx.enter_context(tc.tile_pool(name="ob_pool", bufs=4))

        w_sb = w_pool.tile([P, HID // P, HID], BF16, tag="w")
        nc.sync.dma_start(w_sb[:], w[:, :, :])

        for j in range(NQS):
            ots = []
            for t in range(HID // P):
                tp = psum.tile([P, 512], F32, tag="ps")
                nc.tensor.transpose(
                    tp[:, 0:P], o_acc[:, j, t * P:(t + 1) * P], identity[:],
                )
                ot = ot_pool.tile([P, P], BF16, tag="ot")
                nc.vector.tensor_copy(ot[:], tp[:, 0:P])
                ots.append(ot)
            for h in range(HID // 512):
                pf = psum.tile([P, 512], F32, tag="ps")
                for t in range(HID // P):
                    nc.tensor.matmul(
                        pf[:], ots[t][:], w_sb[:, t, h * 512:(h + 1) * 512],
                        start=(t == 0), stop=(t == HID // P - 1),
                    )
                ob = ob_pool.tile([P, 512], F32, tag="ob")
                nc.scalar.activation(ob[:], pf[:], AF.Relu, scale=rinv[:, j:j + 1])
                rows = min(P, QOUT - j * P)
                if rows > 0:
                    nc.sync.dma_start(
                        out[j * P:j * P + rows, h * 512:(h + 1) * 512], ob[:rows, :],
                    )


def build_program(repeat: int = 0, scratch: int | None = None):
    """Build + compile the SPMD program. repeat>0 wraps the body in a
    device-side For loop (for timing) and is not used for grading."""
    from contextlib import ExitStack

    kw = {} if scratch is None else dict(dynamic_dma_scratch_size=scratch)
    nc = bacc.Bacc("TRN2", target_bir_lowering=False, debug=False, num_devices=8, **kw)

    keysT = nc.dram_tensor("keysT", [D, NKP], F32, kind="ExternalInput")
    qvT = nc.dram_tensor("qvT", [D, QP], F32, kind="ExternalInput")
    adj_r = nc.dram_tensor("adj_r", [P, KT // GRP, 8], I32, kind="ExternalInput")
    adj_s = nc.dram_tensor("adj_s", [P, KT // GRP, 8], I32, kind="ExternalInput")
    src_r = nc.dram_tensor("src_r", [NR, D], BF16, kind="ExternalInput")
    src_s = nc.dram_tensor("src_s", [NS, D], BF16, kind="ExternalInput")
    w = nc.dram_tensor("w", [P, HID // P, HID], BF16, kind="ExternalInput")
    ebias = nc.dram_tensor("ebias", [P, 1], F32, kind="ExternalInput")
    out = nc.dram_tensor("out", [QOUT, HID], F32, kind="ExternalOutput")

    tensors = (keysT, qvT, adj_r, adj_s, src_r, src_s, w, ebias, out)

    with tile.TileContext(nc) as tc, ExitStack() as ctx:
        const = ctx.enter_context(tc.tile_pool(name="const", bufs=1))
        psum = ctx.enter_context(tc.tile_pool(name="psum", bufs=6, space="PSUM"))
        psum_r = ctx.enter_context(tc.tile_pool(name="psum_r", bufs=2, space="PSUM"))
        pools = (const, psum, psum_r)
        # NB: collectives inside a device-side For loop desync the mesh, so
        # the timing variant statically unrolls the body instead.
        for _ in range(max(repeat, 1)):
            _emit_body(nc, tc, pools, tensors)

    nc.compile()
    return nc


def _permute_w(w_full: np.ndarray) -> np.ndarray:
    """Reference cu columns are slot-interleaved [r0 i0 r1 i1 ...]; the kernel
    gathers [r0..r7 | i0..i7]. Permute W rows to match, then pre-tile to
    [128, 8, 1024] for the on-device layout."""
    wr = w_full.reshape(8, 2, D, HID)
    w_perm = np.concatenate(
        [wr[:, 0].reshape(8 * D, HID), wr[:, 1].reshape(8 * D, HID)], axis=0,
    )
    return np.ascontiguousarray(
        w_perm.reshape(HID // P, P, HID).transpose(1, 0, 2),
    )


def _pad_adj(adj: np.ndarray, rank: int) -> np.ndarray:
    """[6000, 8] -> [128, KT//GRP, 8] int32: this core's share of the
    adjacency rows (k-tiles kt = 8*chunk + 2*rank + i), padded rows index
    row 0 (harmless: their attention weight is forced to zero on device)."""
    a = np.zeros((NKP, 8), dtype=np.int32)
    a[:NK] = adj
    tiles = a.reshape(KT, P, 8)
    own = np.stack([tiles[(ci * KC + LOC * rank):(ci * KC + LOC * rank) + LOC]
                    for ci in range(KT // KC)])            # [6, LOC, 128, 8]
    return np.ascontiguousarray(
        own.reshape(KT // GRP, P, 8).transpose(1, 0, 2))


def _host_inputs(review_vecs, user_vecs, item_vecs, user_weights, item_weights,
                 user_review_adj, user_item_adj, item_review_adj, item_user_adj):
    review_vecs = np.asarray(review_vecs, dtype=np.float32)
    user_vecs = np.asarray(user_vecs, dtype=np.float32)
    item_vecs = np.asarray(item_vecs, dtype=np.float32)

    sides = {}
    for side, keys, adj_r, adj_s, src_s, w_full in (
        ("user", user_vecs, user_review_adj, user_item_adj, item_vecs, user_weights),
        ("item", item_vecs, item_review_adj, item_user_adj, user_vecs, item_weights),
    ):
        keysT = np.zeros((D, NKP), dtype=np.float32)
        keysT[:, :NK] = keys.T
        sides[side] = dict(
            keysT=keysT,
            adj_r_full=np.asarray(adj_r, dtype=np.int32),
            adj_s_full=np.asarray(adj_s, dtype=np.int32),
            src_s=np.ascontiguousarray(src_s),
            w=_permute_w(np.asarray(w_full, dtype=np.float32)),
            keys=keys,
        )

    ebias = np.zeros((P, 1), dtype=np.float32)
    ebias[NK - (KT_CALC - 1) * P:] = -1e30

    in_maps = []
    for c in range(8):
        s = sides["user" if c < 4 else "item"]
        b = c % 4
        qv = s["keys"][b * QOUT:(b + 1) * QOUT]  # [1500, 64]
        qvT = np.empty((D, QP), dtype=np.float32)
        qvT[:, :QOUT] = qv.T
        qvT[:, QOUT:] = qv.T[:, :QP - QOUT]  # pad with real vectors (finite rowsums)
        in_maps.append(dict(
            keysT=s["keysT"], qvT=np.ascontiguousarray(qvT),
            adj_r=_pad_adj(s["adj_r_full"], b), adj_s=_pad_adj(s["adj_s_full"], b),
            src_r=review_vecs.astype(ml_dtypes.bfloat16),
            src_s=s["src_s"].astype(ml_dtypes.bfloat16),
            w=s["w"].astype(ml_dtypes.bfloat16), ebias=ebias,
        ))
    return in_maps


_NC_CACHE = None


def kernel(**inputs):
    global _NC_CACHE
    if _NC_CACHE is None:
        _NC_CACHE = build_program()
    nc = _NC_CACHE
    in_maps = _host_inputs(**inputs)
    res = run_bass_kernel_spmd(nc, in_maps, core_ids=list(range(8)))
    outs = [res.results[c]["out"] for c in range(8)]
    user_output = np.concatenate(outs[0:4], axis=0)
    item_output = np.concatenate(outs[4:8], axis=0)
    return user_output, item_output
